# revision 38
# baseline (speedup 1.0000x reference)
"""Trainium2 Bass kernel: batched multi-head attention (v5).

  out = softmax(scale * (Q @ K^T)) @ V    per (batch, head)

Full shapes: Q/K/V [4, 16, 2048, 128] f32, scale [4, 16, 1, 1] f32.
Sharding: 64 batch*head pairs split across 8 NeuronCores (8 heads per
core, no cross-core communication).

Design (evidence: TimelineSim cost model + HW timing; PE-bound):
  - QK^T as single fp16 matmuls (scale folded into Q during the
    f32->f16 GpSimd cast); per 128-row q-chunk: 4 matmuls into three
    PSUM score tiles (512 / 512 / 1024).
  - softmax bias = stride-4 row max of score seg 0 minus a margin.
    Softmax is shift-invariant per row, so any per-row bias gives the
    exact result; only the exp range matters. bf16 P (e^+-88 dynamic
    range) absorbs the subsample gap; row sums come free via the exp
    accum_out, and 1/l is applied at the end.  This keeps DVE reduces
    tiny and off the exp critical path (seg-0 is double-buffered).
  - P^T via PE transposes (bf16) through 1-bank PSUM slots, DVE-copied
    to SBUF; PV with V stationary accumulates O^T in two 1-bank
    PSUM tiles.
  - Software pipelining: each (head, half) unit's PV matmuls ride
    inside the NEXT unit's chunk loop (2 t-chunks per q-chunk); the
    next head's input DMAs issue one unit early (SP queue carries only
    loads; output DMA goes via the GpSimd SWDGE queue) and its prep
    transposes interleave into the preceding unit's chunks.
  - PSUM budget (8 banks): s0(1) + sa(1) + sb(2) + transpose slots
    xT(2) + PV accumulators otA/otB(2).
"""

import numpy as np

import concourse.bass as bass
import concourse.mybir as mybir
import concourse.tile as tile
from concourse import bacc
from concourse.masks import make_identity

B, H, S, D = 4, 16, 2048, 128
N_CORES = 8
HEADS_PER_CORE = (B * H) // N_CORES  # 8

F32 = mybir.dt.float32
F16 = mybir.dt.float16
BF16 = mybir.dt.bfloat16
AX = mybir.AxisListType.X
EXP = mybir.ActivationFunctionType.Exp

# "f16": single fp16 matmul for QK; "x2b": q hi/lo fp16 split (2 matmuls)
QK_MODE = "f16"
P_DTYPE = BF16
MARGIN = 20.0

TRACE = False
LAST_EXEC_NS = None


def _bcast_ap(ap, parts):
    return bass.AP(tensor=ap.tensor, offset=ap.offset, ap=[[0, parts], [1, 1]])


def build_attention_nc(
    n_heads=HEADS_PER_CORE,
    seq=S,
    p_dtype=None,
    qk_mode=None,
    repeat=1,
    ablate=frozenset(),
    rowmax_sub=None,  # unused; kept for test.py compat
):
    import contextlib

    if p_dtype is None:
        p_dtype = P_DTYPE
    if qk_mode is None:
        qk_mode = QK_MODE

    P = 128
    assert seq % P == 0

    nc = bacc.Bacc("TRN2", target_bir_lowering=False)
    q_d = nc.declare_dram_parameter("q", [n_heads, seq, D], F32, isOutput=False)
    k_d = nc.declare_dram_parameter("k", [n_heads, seq, D], F32, isOutput=False)
    v_d = nc.declare_dram_parameter("v", [n_heads, seq, D], F32, isOutput=False)
    s_d = nc.declare_dram_parameter("scale", [n_heads, 1], F32, isOutput=False)
    o_d = nc.declare_dram_parameter("out", [n_heads, seq, D], F32, isOutput=True)

    with tile.TileContext(nc) as tc:
        with (
            tc.tile_pool(name="singles", bufs=1) as singles,
            tc.tile_pool(name="raw", bufs=2) as raw,
            tc.tile_pool(name="cast", bufs=2) as castp,
            tc.tile_pool(name="qkT", bufs=2) as qkT,
            tc.tile_pool(name="prow", bufs=3) as prow,
            tc.tile_pool(name="ptb", bufs=2) as ptb,
            tc.tile_pool(name="stats", bufs=2) as stats,
            tc.tile_pool(name="osb", bufs=2) as osb,
            tc.tile_pool(name="ps", bufs=1, space="PSUM") as ps,
            tc.tile_pool(name="psPV", bufs=1, space="PSUM") as psPV,
        ):
            pools = dict(
                raw=raw, cast=castp, qkT=qkT, prow=prow, ptb=ptb,
                stats=stats, osb=osb, ps=ps, psPV=psPV,
            )
            ident16 = singles.tile([P, P], F16, tag="id16")
            make_identity(nc, ident16)
            identp = singles.tile([P, P], p_dtype, tag="idp")
            make_identity(nc, identp)
            idents = dict(f16=ident16, p=identp)

            rep_ctx = (
                tc.For_i(0, repeat, 1) if repeat > 1 else contextlib.nullcontext()
            )
            with rep_ctx:
                _build_body(
                    nc, n_heads, seq, p_dtype, qk_mode,
                    q_d, k_d, v_d, s_d, o_d, pools, idents, ablate,
                )

    nc.compile()
    return nc


def _build_body(
    nc, n_heads, seq, p_dtype, qk_mode, q_d, k_d, v_d, s_d, o_d, pools, idents, ab,
):
    P = 128
    NQ = seq // P           # 16 q-chunks
    NT = seq // P           # 16 t-chunks
    NH = NQ // 2            # 8 q-chunks per half-unit
    half_s = NH * P         # 1024
    SEG = 512
    x2b = qk_mode == "x2b"

    raw, castp, qkT, prow = pools["raw"], pools["cast"], pools["qkT"], pools["prow"]
    ptb, stats, osb = pools["ptb"], pools["stats"], pools["osb"]
    ps, psPV = pools["ps"], pools["psPV"]
    ident16, identp = idents["f16"], idents["p"]

    def issue_loads(h):
        hc = {}
        scale_b = stats.tile([P, 1], F32, tag="scaleb", name=f"scb_{h}")
        nc.sync.dma_start(out=scale_b, in_=_bcast_ap(s_d[h], P))
        q_raw = raw.tile([P, NQ, D], F32, tag="qraw", name=f"qr_{h}")
        k_raw = raw.tile([P, NT, D], F32, tag="kraw", name=f"kr_{h}")
        v_raw = raw.tile([P, NT, D], F32, tag="vraw", name=f"vr_{h}")
        if "noload" not in ab:
            nc.sync.dma_start(out=q_raw, in_=q_d[h].rearrange("(c p) d -> p c d", p=P))
            nc.sync.dma_start(out=k_raw, in_=k_d[h].rearrange("(c p) d -> p c d", p=P))
            nc.sync.dma_start(out=v_raw, in_=v_d[h].rearrange("(c p) d -> p c d", p=P))
        hc.update(scale_b=scale_b, q_raw=q_raw, k_raw=k_raw, v_raw=v_raw)
        return hc

    def issue_casts(h, hc):
        q_raw, k_raw, v_raw = hc["q_raw"], hc["k_raw"], hc["v_raw"]
        q16 = castp.tile([P, NQ, D], F16, tag="q16", name=f"q16_{h}")
        k16 = castp.tile([P, NT, D], F16, tag="k16", name=f"k16_{h}")
        v16 = castp.tile([P, NT, D], p_dtype, tag="v16", name=f"v16_{h}")
        nc.gpsimd.tensor_scalar_mul(out=q16, in0=q_raw, scalar1=hc["scale_b"])
        nc.gpsimd.tensor_copy(out=k16, in_=k_raw)
        nc.gpsimd.tensor_copy(out=v16, in_=v_raw)
        srcs = [(q16, "qT"), (k16, "kT")]
        if x2b:
            qlo = castp.tile([P, NQ, D], F16, tag="qlo", name=f"qlo_{h}")
            qsc = castp.tile([P, NQ, D], F32, tag="qsc", name=f"qsc_{h}")
            nc.vector.tensor_scalar_mul(out=qsc, in0=q_raw, scalar1=hc["scale_b"])
            nc.vector.tensor_sub(out=qlo, in0=qsc, in1=q16)
            srcs.append((qlo, "qloT"))
        hc["v16"] = v16
        hc["lp"] = stats.tile([P, NQ, 3], F32, tag="lp", name=f"lp_{h}")
        hc["rl"] = stats.tile([P, NQ], F32, tag="rl", name=f"rl_{h}")

        # prep transposes as deferred closures (interleaved into the
        # preceding unit's chunk loop)
        closures = []
        for src, nm in srcs:
            dst = qkT.tile([P, seq], F16, tag=nm, name=f"{nm}_{h}")
            hc[nm] = dst
            if "prep" in ab:
                continue
            for g0 in (0, 8):

                def cl(src=src, dst=dst, g0=g0, h=h, nm=nm):
                    tp = ps.tile([P, SEG], F32, tag="xT", bufs=2,
                                 name=f"prep_{h}_{nm}_{g0}")
                    tp16 = tp.bitcast(F16)
                    for j in range(8):
                        nc.tensor.transpose(
                            tp16[:, j * P : (j + 1) * P], src[:, g0 + j, :],
                            ident16,
                        )
                    nc.vector.tensor_copy(
                        out=dst[:, g0 * P : (g0 + 8) * P], in_=tp16
                    )

                closures.append(cl)
        return closures

    def finalize(pu, otA, otB):
        # O^T -> O, normalize by 1/l, store (for the unit that just
        # finished its PV accumulation)
        ph, phalf, ppT, phc = pu
        pqoff = phalf * NH
        oT_sb = osb.tile([P, half_s], p_dtype, tag="otsb", name=f"ots_{ph}_{phalf}")
        nc.vector.tensor_copy(out=oT_sb[:, 0:SEG], in_=otA)
        nc.vector.tensor_copy(out=oT_sb[:, SEG:], in_=otB)
        o_sb = osb.tile([P, NH, D], F32, tag="osb", name=f"osb_{ph}_{phalf}")
        if "dtrans" not in ab:
            lsum = stats.tile([P, NH], F32, tag="lsum", name=f"ls_{ph}_{phalf}")
            nc.vector.reduce_sum(lsum, phc["lp"][:, pqoff : pqoff + NH, :], axis=AX)
            nc.vector.reciprocal(phc["rl"][:, pqoff : pqoff + NH], lsum)
            for g in range(2):
                tp = ps.tile([P, SEG], F32, tag="xT", bufs=2,
                             name=f"od_{ph}_{phalf}_{g}")
                tpv = tp.bitcast(p_dtype)
                for j in range(4):
                    qq = g * 4 + j
                    nc.tensor.transpose(
                        tpv[:, j * P : (j + 1) * P],
                        oT_sb[:, qq * P : (qq + 1) * P],
                        identp,
                    )
                for j in range(4):
                    qq = g * 4 + j
                    nc.vector.tensor_scalar_mul(
                        out=o_sb[:, qq, :],
                        in0=tpv[:, j * P : (j + 1) * P],
                        scalar1=phc["rl"][:, pqoff + qq : pqoff + qq + 1],
                    )
        else:
            nc.gpsimd.memset(o_sb, 0.0)
        # out-DMA on the gpsimd (SWDGE) queue so the SP queue stays
        # dedicated to input prefetch
        nc.gpsimd.dma_start(
            out=o_d[ph].rearrange("(c p) d -> p c d", p=P)[
                :, pqoff : pqoff + NH, :
            ],
            in_=o_sb,
        )

    heads = {}
    heads[0] = issue_loads(0)
    prep_pend = issue_casts(0, heads[0])
    for cl in prep_pend:
        cl()
    prep_pend = []
    prev = None  # (h, half, pT, hc) whose PV is issued during this unit

    for ui in range(2 * n_heads + 1):
        flush = ui == 2 * n_heads
        if not flush:
            h, half = divmod(ui, 2)
            if half == 0 and h + 1 < n_heads:
                # prefetch next head's inputs (SP queue carries loads only)
                heads[h + 1] = issue_loads(h + 1)
            if half == 1 and h + 1 < n_heads:
                # next head's casts now; its prep transposes interleave
                # into this unit's chunk loop below
                prep_pend = issue_casts(h + 1, heads[h + 1])
            hc = heads[h]
            qT, kT = hc["qT"], hc["kT"]
            qloT = hc.get("qloT")
            lp = hc["lp"]
            qoff = half * NH
            pT = ptb.tile([P, NT, half_s], p_dtype, tag="pT", name=f"pT_{ui}")
            if h > 1 and half == 0 and (h - 2) in heads:
                del heads[h - 2]

        if prev is not None and "pv" not in ab:
            otA = psPV.tile([P, SEG], F32, tag="otA", name=f"otA_{ui}")
            otB = psPV.tile([P, SEG], F32, tag="otB", name=f"otB_{ui}")
            pv16, ppT = prev[3]["v16"], prev[2]

        def pv_pair(tc_i):
            nc.tensor.matmul(
                otA, pv16[:, tc_i, :], ppT[:, tc_i, 0:SEG],
                start=(tc_i == 0), stop=(tc_i == NT - 1),
            )
            nc.tensor.matmul(
                otB, pv16[:, tc_i, :], ppT[:, tc_i, SEG:],
                start=(tc_i == 0), stop=(tc_i == NT - 1),
            )

        for qq in range(0 if flush else NH):
            qi = qoff + qq
            qs = slice(qi * P, (qi + 1) * P)

            # ---- scores: s0 / sa (512) + sb (1024) -----------------
            # exp bias = strided row max of seg 0 (shift-invariant;
            # margin keeps exp in range, bf16 P absorbs the gap)
            st0 = ps.tile([P, SEG], F32, tag="s0", name=f"s0_{ui}_{qi}")
            sta = ps.tile([P, SEG], F32, tag="sa", name=f"sa_{ui}_{qi}")
            stb = ps.tile([P, 2 * SEG], F32, tag="sb", name=f"sb_{ui}_{qi}")
            segs = [(st0, 0), (sta, SEG), (stb, 2 * SEG)]

            negm = stats.tile([P, 1], F32, tag="negm", name=f"negm_{ui}_{qi}")
            if "qk" not in ab:
                passes = [(qT, True, not x2b)] + (
                    [(qloT, False, True)] if x2b else []
                )
                for mat, st_flag, sp_flag in passes:
                    for stt, off in segs:
                        w = stt.shape[-1]
                        for jo in range(0, w, SEG):
                            nc.tensor.matmul(
                                stt[:, jo : jo + SEG], mat[:, qs],
                                kT[:, off + jo : off + jo + SEG],
                                start=st_flag, stop=sp_flag,
                            )
            if "reduce" not in ab:
                st0v = st0.rearrange("p (a b) -> p a b", b=4)[:, :, 0]
                nc.vector.reduce_max(negm, st0v, axis=AX, negate=True)
                nc.vector.tensor_scalar_sub(out=negm, in0=negm, scalar1=MARGIN)

            # ---- exp (+ row-sum accumulation) -----------------------
            p_row = prow.tile([P, seq], p_dtype, tag="prow", name=f"pr_{ui}_{qi}")
            if "exp" not in ab:
                for jj, (stt, off) in enumerate(segs):
                    w = stt.shape[-1]
                    nc.scalar.activation(
                        out=p_row[:, off : off + w], in_=stt, func=EXP,
                        bias=negm, accum_out=lp[:, qi, jj : jj + 1],
                    )

            # ---- previous unit's PV rides along ---------------------
            if prev is not None and "pv" not in ab:
                pv_pair(2 * qq)
                pv_pair(2 * qq + 1)

            # ---- next head's prep transposes ride along -------------
            if prep_pend and qq >= 2:
                prep_pend.pop(0)()

            # ---- P^T: PE transposes + PSUM->SBUF copies -------------
            if "ptrans" not in ab:
                for g in range(2):
                    tp = ps.tile([P, SEG], F32, tag="xT", bufs=2,
                                 name=f"pt_{ui}_{qi}_{g}")
                    tpv = tp.bitcast(p_dtype)
                    for j in range(8):
                        tck = g * 8 + j
                        nc.tensor.transpose(
                            tpv[:, j * P : (j + 1) * P],
                            p_row[:, tck * P : (tck + 1) * P],
                            identp,
                        )
                    if "pcopy" not in ab:
                        dst = pT[:, g * 8 : g * 8 + 8, qq * P : (qq + 1) * P]
                        srcv = tpv.rearrange("p (a b) -> p a b", a=8)
                        nc.vector.tensor_copy(out=dst, in_=srcv)

        while prep_pend:
            prep_pend.pop(0)()
        if flush and prev is not None and "pv" not in ab:
            for tc_i in range(NT):
                pv_pair(tc_i)
        if prev is not None and "pv" not in ab:
            finalize(prev, otA, otB)
        prev = None if flush else (h, half, pT, hc)


_NC_CACHE = {}


def _get_nc():
    key = (HEADS_PER_CORE, S, P_DTYPE, QK_MODE)
    if key not in _NC_CACHE:
        _NC_CACHE[key] = build_attention_nc()
    return _NC_CACHE[key]


def kernel(query, key, value, scale_factor):
    global LAST_EXEC_NS
    from concourse.bass_utils import run_bass_kernel_spmd

    q = np.ascontiguousarray(np.asarray(query, dtype=np.float32).reshape(B * H, S, D))
    k = np.ascontiguousarray(np.asarray(key, dtype=np.float32).reshape(B * H, S, D))
    v = np.ascontiguousarray(np.asarray(value, dtype=np.float32).reshape(B * H, S, D))
    sc = np.ascontiguousarray(
        np.asarray(scale_factor, dtype=np.float32).reshape(B * H, 1)
    )

    nc = _get_nc()
    in_maps = []
    for c in range(N_CORES):
        sl = slice(c * HEADS_PER_CORE, (c + 1) * HEADS_PER_CORE)
        in_maps.append({"q": q[sl], "k": k[sl], "v": v[sl], "scale": sc[sl]})

    res = run_bass_kernel_spmd(nc, in_maps, list(range(N_CORES)), trace=TRACE)
    LAST_EXEC_NS = res.exec_time_ns
    outs = [np.asarray(res.results[c]["out"]) for c in range(N_CORES)]
    return np.concatenate(outs, axis=0).reshape(B, H, S, D).astype(np.float32)
